# revision 1
# baseline (speedup 1.0000x reference)
"""Cosine-similarity attention kernel for Trainium2 (8 NeuronCores, SPMD).

Problem: B=4, D=1024, T=2048, n_head=8, alpha=5.0.
Math (per batch b, head h, with d = D/8 = 128):
    qn = l2norm(q, axis=d); kn = l2norm(k, axis=d)
    S  = alpha * qn^T kn          [Tq, Tk]
    P  = softmax(S, axis=Tk)
    out= v @ P^T                  [dv, Tq]

Sharding: head-parallel — the 32 (b, h) pairs are split 4-per-core across
8 cores. Each core computes full attention for its 4 pairs.

Device-side design notes:
  - Scores are computed TRANSPOSED (S^T = kn^T @ qn, [k, q] layout) so the
    AV matmul contracts over k (the partition dim) without transposing the
    softmax matrix. Softmax max-subtraction is skipped: |S| <= alpha = 5,
    so exp is in [e-5, e5] — safe in fp32.
  - l2norm: sum-of-squares over the partition dim via an all-ones matmul,
    then rsqrt as exp(-0.5*ln(x)) on the scalar engine (Ln and Exp share
    one activation-table set; Rsqrt is rejected by bass for accuracy).
  - Row sums of exp(S^T) via an all-ones matmul accumulated over k-tiles;
    the softmax divide is exp(-ln(rowsum)) broadcast from the same psum.
  - Matmuls run as float32r (fp32 data at 1 column/cycle vs 4 for plain
    fp32 on TRN2).
"""

import math
import os
import sys
from contextlib import ExitStack

for _p in ("/opt/trn_rl_repo", "/root/.axon_site/_ro/trn_rl_repo"):
    if os.path.isdir(_p) and _p not in sys.path:
        sys.path.insert(0, _p)

import numpy as np

import concourse.bass as bass
import concourse.tile as tile
from concourse import bacc, mybir
from concourse.bass_utils import run_bass_kernel_spmd

N_CORES = 8
B, DFULL, T = 4, 1024, 2048
NHEAD = 8
D = DFULL // NHEAD          # 128 per-head channels
PAIRS = (B * NHEAD) // N_CORES  # 4 (b, h) pairs per core
ALPHA = 5.0

NKT = T // 128              # 16 k-tiles of 128
QB = 512                    # q-block width
NQB = T // QB               # 4 q-blocks
CK = 2                      # k-tiles per exp chunk ([128, CK*512] psum chunk)

F32 = mybir.dt.float32
F32R = mybir.dt.float32r
EXP = mybir.ActivationFunctionType.Exp
LN = mybir.ActivationFunctionType.Ln


class _PinnedActBacc(bacc.Bacc):
    """Bacc whose activation-table chooser is pinned so Exp and Ln both
    resolve to natural_log_exp_and_others. The default chooser maps Exp to
    exp_and_others and Ln to natural_log, inserting a ~1.3-2.7us table load
    at every Ln/Exp alternation (48 loads in this kernel)."""

    def insert_act_table_loads(self):
        import bass_rust as _bass_rust
        from concourse.hw_specs import get_activation_tables

        has_activation = any(
            isinstance(i, mybir.InstActivation)
            for b in self.main_func.blocks
            for i in b.instructions
        )
        if not has_activation:
            return
        keep = "natural_log_exp_and_others"
        drop = {
            mybir.ActivationFunctionType.Exp,
            mybir.ActivationFunctionType.Ln,
        }
        tables = []
        for name, fns in get_activation_tables(self.m.arch).items():
            tables.append((name, fns if name == keep else (fns - drop)))
        _bass_rust.insert_act_table_loads(self, tables)


def _build_nc(repeat: int = 1) -> bass.Bass:
    nc = _PinnedActBacc(None, target_bir_lowering=False)
    q_d = nc.declare_dram_parameter("q", [PAIRS, D, T], F32, isOutput=False)
    k_d = nc.declare_dram_parameter("k", [PAIRS, D, T], F32, isOutput=False)
    vt_d = nc.declare_dram_parameter("vt", [PAIRS, T, D], F32R, isOutput=False)
    out_d = nc.declare_dram_parameter("out", [PAIRS, D, T], F32, isOutput=True)

    with ExitStack() as ctx:
        tc = ctx.enter_context(tile.TileContext(nc))
        const_p = ctx.enter_context(tc.tile_pool(name="const", bufs=1))
        io_p = ctx.enter_context(tc.tile_pool(name="io", bufs=2))
        work_p = ctx.enter_context(tc.tile_pool(name="work", bufs=2))
        e_p = ctx.enter_context(tc.tile_pool(name="e", bufs=12))
        out_p = ctx.enter_context(tc.tile_pool(name="outp", bufs=3))
        # PSUM: chunk pool 2x[128,1024] (4 banks) + av 2x[128,512] (2 banks)
        # + rowsum 2x[128,512] (2 banks) = 8 banks.
        cps = ctx.enter_context(tc.tile_pool(name="cps", bufs=3, space="PSUM"))
        avps = ctx.enter_context(tc.tile_pool(name="avps", bufs=1, space="PSUM"))
        rsps = ctx.enter_context(tc.tile_pool(name="rsps", bufs=1, space="PSUM"))

        ones_f32 = const_p.tile([128, 128], F32)
        nc.vector.memset(ones_f32, 1.0)
        ones = const_p.tile([128, 128], F32R)
        nc.vector.tensor_copy(ones, ones_f32)
        # per-partition bias tile holding 0.5*ln(alpha) (activation bias
        # must be an SBUF AP for non-trivial constants)
        bias_hla = const_p.tile([128, 1], F32)
        nc.vector.memset(bias_hla, 0.5 * math.log(ALPHA))

        def emit_load(p):
            q_sb = io_p.tile([D, T], F32, tag="q")
            k_sb = io_p.tile([D, T], F32, tag="k")
            vt_sb = io_p.tile([128, NKT, D], F32R, tag="vt")
            nc.sync.dma_start(out=q_sb, in_=q_d[p])
            nc.sync.dma_start(out=k_sb, in_=k_d[p])
            # vt dram [T, D] -> sbuf [128, kt, dv]: partition = k % 128
            nc.sync.dma_start(
                out=vt_sb,
                in_=vt_d[p].rearrange("(t kp) dv -> kp t dv", kp=128),
            )
            return q_sb, k_sb, vt_sb

        # ---- cosine normalization: xn = sqrt(alpha) * x / ||x||_d ----
        def emit_norms(q_sb, k_sb):
            qn = work_p.tile([D, T], F32R, tag="qn")
            kn = work_p.tile([D, T], F32R, tag="kn")
            for src, dst in ((q_sb, qn), (k_sb, kn)):
                sq = work_p.tile([D, T], F32R, tag="sq")
                nc.vector.tensor_mul(sq, src, src)
                for hh in range(2):
                    ssq = cps.tile([128, 1024], F32, tag="chunk")
                    for j in range(2):
                        off = hh * 1024 + j * 512
                        nc.tensor.matmul(
                            ssq[:, j * 512:(j + 1) * 512],
                            lhsT=ones,
                            rhs=sq[:, off:off + 512],
                            start=True, stop=True,
                        )
                    lnt = work_p.tile([128, 1024], F32, tag="lnt")
                    nc.scalar.activation(lnt, ssq, LN)
                    inv = work_p.tile([128, 1024], F32, tag="inv")
                    nc.scalar.activation(
                        inv, lnt, EXP, scale=-0.5, bias=bias_hla
                    )
                    sl = slice(hh * 1024, (hh + 1) * 1024)
                    nc.vector.tensor_mul(dst[:, sl], src[:, sl], inv)
            return qn, kn

        # software pipeline across pairs: pair p+1's loads and norms are
        # emitted between pair p's q-blocks so their DVE/ACT/PE work fills
        # the scheduler's pair-boundary bubble
        total = PAIRS * repeat
        cur_load = emit_load(0)
        cur_norm = emit_norms(cur_load[0], cur_load[1])
        nxt_load = nxt_norm = None
        for p_rep in range(total):
            p = p_rep % PAIRS
            qn, kn = cur_norm
            vt_sb = cur_load[2]
            for qb in range(NQB):
                qsl = slice(qb * QB, (qb + 1) * QB)
                av = avps.tile([128, QB], F32, tag="av")
                rs = rsps.tile([128, QB], F32, tag="rs")
                for c in range(NKT // CK):
                    sp = cps.tile([128, CK * 512], F32, tag="chunk")
                    for j in range(CK):
                        kt = CK * c + j
                        nc.tensor.matmul(
                            sp[:, j * 512:(j + 1) * 512],
                            lhsT=kn[:, kt * 128:(kt + 1) * 128],
                            rhs=qn[:, qsl],
                            start=True, stop=True,
                        )
                    e_c = e_p.tile([128, CK * 512], F32R, tag="e")
                    nc.scalar.activation(e_c, sp, EXP)
                    for j in range(CK):
                        kt = CK * c + j
                        e_sl = e_c[:, j * 512:(j + 1) * 512]
                        nc.tensor.matmul(
                            av,
                            lhsT=vt_sb[:, kt, :],
                            rhs=e_sl,
                            start=(kt == 0), stop=(kt == NKT - 1),
                        )
                        # row sum over k rides along: all-ones matmul gives
                        # the rowsum broadcast across all 128 psum rows
                        nc.tensor.matmul(
                            rs,
                            lhsT=ones,
                            rhs=e_sl,
                            start=(kt == 0), stop=(kt == NKT - 1),
                        )
                invr = out_p.tile([128, QB], F32, tag="invr")
                nc.vector.reciprocal_approx_fast(out=invr, in_=rs)
                o_sb = out_p.tile([128, QB], F32, tag="o")
                nc.vector.tensor_mul(o_sb, av, invr)
                nc.sync.dma_start(out=out_d[p][:, qsl], in_=o_sb)
                if p_rep + 1 < total:
                    if qb == 0:
                        nxt_load = emit_load((p_rep + 1) % PAIRS)
                    elif qb == 1:
                        nxt_norm = emit_norms(nxt_load[0], nxt_load[1])
            if p_rep + 1 < total:
                cur_load, cur_norm = nxt_load, nxt_norm

    nc.finalize()
    return nc


_NC_CACHE = None


def _get_nc() -> bass.Bass:
    global _NC_CACHE
    if _NC_CACHE is None:
        _NC_CACHE = _build_nc()
    return _NC_CACHE


def make_in_maps(q: np.ndarray, k: np.ndarray, v: np.ndarray):
    """Shard full [B, D, T] inputs into per-core in_maps."""
    qr = q.reshape(B * NHEAD, D, T)
    kr = k.reshape(B * NHEAD, D, T)
    vr = v.reshape(B * NHEAD, D, T).transpose(0, 2, 1)  # [32, T, d]
    in_maps = []
    for c in range(N_CORES):
        sl = slice(c * PAIRS, (c + 1) * PAIRS)
        in_maps.append({
            "q": np.ascontiguousarray(qr[sl], dtype=np.float32),
            "k": np.ascontiguousarray(kr[sl], dtype=np.float32),
            "vt": np.ascontiguousarray(vr[sl], dtype=np.float32),
        })
    return in_maps


def gather_out(results) -> np.ndarray:
    outs = np.concatenate(
        [results[c]["out"] for c in range(N_CORES)], axis=0
    )  # [32, d, T]
    return np.ascontiguousarray(outs.reshape(B, DFULL, T), dtype=np.float32)


def run(q, k, v, **kwargs):
    nc = _get_nc()
    res = run_bass_kernel_spmd(nc, make_in_maps(q, k, v), list(range(N_CORES)), **kwargs)
    return gather_out(res.results), res


def kernel(q: np.ndarray, k: np.ndarray, v: np.ndarray) -> np.ndarray:
    out, _ = run(q, k, v)
    return out



# revision 2
# speedup vs baseline: 1.5990x; 1.5990x over previous
"""Cosine-similarity attention kernel for Trainium2 (8 NeuronCores, SPMD).

Problem: B=4, D=1024, T=2048, n_head=8, alpha=5.0.
Math (per batch b, head h, with d = D/8 = 128):
    qn = l2norm(q, axis=d); kn = l2norm(k, axis=d)
    S  = alpha * qn^T kn          [Tq, Tk]
    P  = softmax(S, axis=Tk)
    out= v @ P^T                  [dv, Tq]

Sharding: head-parallel — the 32 (b, h) pairs are split 4-per-core across
8 cores. Each core computes full attention for its 4 pairs.

Device-side design notes:
  - Scores are computed TRANSPOSED (S^T = kn^T @ qn, [k, q] layout) so the
    AV matmul contracts over k (the partition dim) without transposing the
    softmax matrix. Softmax max-subtraction is skipped: |S| <= alpha = 5,
    so exp is in [e-5, e5] — safe in fp32.
  - l2norm: sum-of-squares over the partition dim via an all-ones matmul,
    then rsqrt as exp(-0.5*ln(x)) on the scalar engine (Ln and Exp share
    one activation-table set; Rsqrt is rejected by bass for accuracy).
  - Row sums of exp(S^T) via an all-ones matmul accumulated over k-tiles;
    the softmax divide is a reciprocal_approx_fast + multiply on DVE.
  - Matmuls run as float32r (fp32 data at full stream rate vs 4 cycles/col
    for plain fp32 on TRN2).
  - q-block software pipeline: QK matmuls + exp of q-block i+1 are emitted
    BEFORE the AV/rowsum matmuls of q-block i (globally, across pair
    boundaries), staging a full q-block of exp(S^T) tiles in SBUF. This
    gives the scalar-engine exp a whole block (~8 chunks) of slack, so the
    PE never waits on a just-issued exp — measured ~10-50us faster than
    chunk-granular interleaving depending on machine load.
"""

import math
import os
import sys
from contextlib import ExitStack

for _p in ("/opt/trn_rl_repo", "/root/.axon_site/_ro/trn_rl_repo"):
    if os.path.isdir(_p) and _p not in sys.path:
        sys.path.insert(0, _p)

import numpy as np

import concourse.bass as bass
import concourse.tile as tile
from concourse import bacc, mybir
from concourse.bass_utils import run_bass_kernel_spmd

N_CORES = 8
B, DFULL, T = 4, 1024, 2048
NHEAD = 8
D = DFULL // NHEAD          # 128 per-head channels
PAIRS = (B * NHEAD) // N_CORES  # 4 (b, h) pairs per core
ALPHA = 5.0

NKT = T // 128              # 16 k-tiles of 128
QB = 512                    # q-block width
NQB = T // QB               # 4 q-blocks
CK = 2                      # k-tiles per exp chunk ([128, CK*512] psum chunk)

F32 = mybir.dt.float32
F32R = mybir.dt.float32r
EXP = mybir.ActivationFunctionType.Exp
LN = mybir.ActivationFunctionType.Ln


class _PinnedActBacc(bacc.Bacc):
    """Bacc whose activation-table chooser is pinned so Exp and Ln both
    resolve to natural_log_exp_and_others. The default chooser maps Exp to
    exp_and_others and Ln to natural_log, inserting a ~1.3-2.7us table load
    at every Ln/Exp alternation (48 loads in this kernel)."""

    def insert_act_table_loads(self):
        import bass_rust as _bass_rust
        from concourse.hw_specs import get_activation_tables

        has_activation = any(
            isinstance(i, mybir.InstActivation)
            for b in self.main_func.blocks
            for i in b.instructions
        )
        if not has_activation:
            return
        keep = "natural_log_exp_and_others"
        drop = {
            mybir.ActivationFunctionType.Exp,
            mybir.ActivationFunctionType.Ln,
        }
        tables = []
        for name, fns in get_activation_tables(self.m.arch).items():
            tables.append((name, fns if name == keep else (fns - drop)))
        _bass_rust.insert_act_table_loads(self, tables)


def _build_nc(repeat: int = 1) -> bass.Bass:
    nc = _PinnedActBacc(None, target_bir_lowering=False)
    q_d = nc.declare_dram_parameter("q", [PAIRS, D, T], F32, isOutput=False)
    k_d = nc.declare_dram_parameter("k", [PAIRS, D, T], F32, isOutput=False)
    vt_d = nc.declare_dram_parameter("vt", [PAIRS, T, D], F32R, isOutput=False)
    out_d = nc.declare_dram_parameter("out", [PAIRS, D, T], F32, isOutput=True)

    with ExitStack() as ctx:
        tc = ctx.enter_context(tile.TileContext(nc))
        const_p = ctx.enter_context(tc.tile_pool(name="const", bufs=1))
        io_p = ctx.enter_context(tc.tile_pool(name="io", bufs=2))
        work_p = ctx.enter_context(tc.tile_pool(name="work", bufs=2))
        e_p = ctx.enter_context(tc.tile_pool(name="e", bufs=18))
        out_p = ctx.enter_context(tc.tile_pool(name="outp", bufs=3))
        # PSUM: chunk pool 3x[128,1024] (6 banks) + av [128,512] (1 bank)
        # + rowsum [128,512] (1 bank) = 8 banks.
        cps = ctx.enter_context(tc.tile_pool(name="cps", bufs=3, space="PSUM"))
        avps = ctx.enter_context(tc.tile_pool(name="avps", bufs=1, space="PSUM"))
        rsps = ctx.enter_context(tc.tile_pool(name="rsps", bufs=1, space="PSUM"))

        ones_f32 = const_p.tile([128, 128], F32)
        nc.vector.memset(ones_f32, 1.0)
        ones = const_p.tile([128, 128], F32R)
        nc.vector.tensor_copy(ones, ones_f32)
        # per-partition bias tile holding 0.5*ln(alpha) (activation bias
        # must be an SBUF AP for non-trivial constants)
        bias_hla = const_p.tile([128, 1], F32)
        nc.vector.memset(bias_hla, 0.5 * math.log(ALPHA))

        def emit_load(p):
            q_sb = io_p.tile([D, T], F32, tag="q")
            k_sb = io_p.tile([D, T], F32, tag="k")
            vt_sb = io_p.tile([128, NKT, D], F32R, tag="vt")
            nc.sync.dma_start(out=q_sb, in_=q_d[p])
            nc.sync.dma_start(out=k_sb, in_=k_d[p])
            # vt dram [T, D] -> sbuf [128, kt, dv]: partition = k % 128
            nc.sync.dma_start(
                out=vt_sb,
                in_=vt_d[p].rearrange("(t kp) dv -> kp t dv", kp=128),
            )
            return q_sb, k_sb, vt_sb

        # ---- cosine normalization: xn = sqrt(alpha) * x / ||x||_d ----
        def emit_norms(q_sb, k_sb):
            qn = work_p.tile([D, T], F32R, tag="qn")
            kn = work_p.tile([D, T], F32R, tag="kn")
            for src, dst in ((q_sb, qn), (k_sb, kn)):
                sq = work_p.tile([D, T], F32R, tag="sq")
                nc.vector.tensor_mul(sq, src, src)
                for hh in range(2):
                    ssq = cps.tile([128, 1024], F32, tag="chunk")
                    for j in range(2):
                        off = hh * 1024 + j * 512
                        nc.tensor.matmul(
                            ssq[:, j * 512:(j + 1) * 512],
                            lhsT=ones,
                            rhs=sq[:, off:off + 512],
                            start=True, stop=True,
                        )
                    lnt = work_p.tile([128, 1024], F32, tag="lnt")
                    nc.scalar.activation(lnt, ssq, LN)
                    inv = work_p.tile([128, 1024], F32, tag="inv")
                    nc.scalar.activation(
                        inv, lnt, EXP, scale=-0.5, bias=bias_hla
                    )
                    sl = slice(hh * 1024, (hh + 1) * 1024)
                    nc.vector.tensor_mul(dst[:, sl], src[:, sl], inv)
            return qn, kn

        def emit_qkexp(qn, kn, qsl):
            e_tiles = []
            for c in range(NKT // CK):
                sp = cps.tile([128, CK * 512], F32, tag="chunk")
                for j in range(CK):
                    kt = CK * c + j
                    nc.tensor.matmul(
                        sp[:, j * 512:(j + 1) * 512],
                        lhsT=kn[:, kt * 128:(kt + 1) * 128],
                        rhs=qn[:, qsl],
                        start=True, stop=True,
                    )
                e_c = e_p.tile([128, CK * 512], F32R, tag="e")
                nc.scalar.activation(e_c, sp, EXP)
                e_tiles.append(e_c)
            return e_tiles

        def emit_avrs(pend):
            p, qsl, e_tiles, vt_sb = pend
            av = avps.tile([128, QB], F32, tag="av")
            rs = rsps.tile([128, QB], F32, tag="rs")
            for c in range(NKT // CK):
                for j in range(CK):
                    kt = CK * c + j
                    e_sl = e_tiles[c][:, j * 512:(j + 1) * 512]
                    nc.tensor.matmul(
                        av, lhsT=vt_sb[:, kt, :], rhs=e_sl,
                        start=(kt == 0), stop=(kt == NKT - 1),
                    )
                    # row sum over k rides along: all-ones matmul gives the
                    # rowsum broadcast across all 128 psum rows
                    nc.tensor.matmul(
                        rs, lhsT=ones, rhs=e_sl,
                        start=(kt == 0), stop=(kt == NKT - 1),
                    )
            invr = out_p.tile([128, QB], F32, tag="invr")
            nc.vector.reciprocal_approx_fast(out=invr, in_=rs)
            o_sb = out_p.tile([128, QB], F32, tag="o")
            nc.vector.tensor_mul(o_sb, av, invr)
            nc.sync.dma_start(out=out_d[p][:, qsl], in_=o_sb)

        # software pipeline: pair p+1's loads and norms are emitted between
        # pair p's q-blocks; QK+exp of q-block i+1 precede AV/RS of q-block
        # i globally, so the exp chain always has a block of lookahead.
        total = PAIRS * repeat
        cur_load = emit_load(0)
        cur_norm = emit_norms(cur_load[0], cur_load[1])
        nxt_load = nxt_norm = None
        pending = None
        for p_rep in range(total):
            p = p_rep % PAIRS
            qn, kn = cur_norm
            vt_sb = cur_load[2]
            for qb in range(NQB):
                qsl = slice(qb * QB, (qb + 1) * QB)
                e_tiles = emit_qkexp(qn, kn, qsl)
                if pending is not None:
                    emit_avrs(pending)
                pending = (p, qsl, e_tiles, vt_sb)
                if p_rep + 1 < total:
                    if qb == 0:
                        nxt_load = emit_load((p_rep + 1) % PAIRS)
                    elif qb == 1:
                        nxt_norm = emit_norms(nxt_load[0], nxt_load[1])
            if p_rep + 1 < total:
                cur_load, cur_norm = nxt_load, nxt_norm
        emit_avrs(pending)

    nc.finalize()
    return nc


_NC_CACHE = None


def _get_nc() -> bass.Bass:
    global _NC_CACHE
    if _NC_CACHE is None:
        _NC_CACHE = _build_nc()
    return _NC_CACHE


def make_in_maps(q: np.ndarray, k: np.ndarray, v: np.ndarray):
    """Shard full [B, D, T] inputs into per-core in_maps."""
    qr = q.reshape(B * NHEAD, D, T)
    kr = k.reshape(B * NHEAD, D, T)
    vr = v.reshape(B * NHEAD, D, T).transpose(0, 2, 1)  # [32, T, d]
    in_maps = []
    for c in range(N_CORES):
        sl = slice(c * PAIRS, (c + 1) * PAIRS)
        in_maps.append({
            "q": np.ascontiguousarray(qr[sl], dtype=np.float32),
            "k": np.ascontiguousarray(kr[sl], dtype=np.float32),
            "vt": np.ascontiguousarray(vr[sl], dtype=np.float32),
        })
    return in_maps


def gather_out(results) -> np.ndarray:
    outs = np.concatenate(
        [results[c]["out"] for c in range(N_CORES)], axis=0
    )  # [32, d, T]
    return np.ascontiguousarray(outs.reshape(B, DFULL, T), dtype=np.float32)


def run(q, k, v, **kwargs):
    nc = _get_nc()
    res = run_bass_kernel_spmd(nc, make_in_maps(q, k, v), list(range(N_CORES)), **kwargs)
    return gather_out(res.results), res


def kernel(q: np.ndarray, k: np.ndarray, v: np.ndarray) -> np.ndarray:
    out, _ = run(q, k, v)
    return out
